# revision 4
# baseline (speedup 1.0000x reference)
"""nn_Compressor Trainium2 kernel — self-contained.

Strategy (8 cores, data-parallel over batch x sequence-half, 2048 tok/core):
  kv/sc = x @ [wkv|wgate] via f32r matmuls (full PE rate), accumulated in
  SBUF over 4 K-blocks; the windowed softmax is restructured as two 4-tap
  box filters over e=exp(sc) and g=e*(kv+ape) (same normalizer within a
  window), then fold, RMSNorm, RoPE in token-major layout after a PE
  transpose. Halo kv/sc rows (3 tokens at the shard boundary) and all
  layout permutations are staged on host.
"""
import numpy as np
from contextlib import ExitStack

B, S, DIM = 4, 4096, 4096
HD, RD = 512, 64
RATIO = 4
EPS = 1e-6
BLOCK_SIZE = 64
N_COMP = S // RATIO
T = 2048            # tokens per core
ACCW = 2052         # 3 margin + 2048 + 1 pad (div by 4 for ape pattern)
NEG = -60.0
NCORES = 8

_RUNNER = [None]


# ---------------------------------------------------------------- walrus fix
def _split_sync_waits(bir_json, cap=1):
    """This walrus build accepts only 1 sem-wait per instruction; move the
    excess onto NoOps inserted before the instruction on the same engine."""
    import orjson
    m = orjson.loads(bir_json)
    counter = [0]
    changed = False
    for f in m.get("functions", []):
        for blk in f.get("blocks", []):
            out = []
            for inst in blk.get("instructions", []):
                si = inst.get("sync_info")
                waits = (si.get("on_wait") or []) if si else []
                if len(waits) > cap:
                    extra = waits[:-cap]
                    si["on_wait"] = waits[-cap:]
                    for i in range(0, len(extra), cap):
                        counter[0] += 1
                        out.append({
                            "name": f"I-wsplit-{counter[0]}",
                            "opcode": "NoOp",
                            "engine": inst["engine"],
                            "ins": [], "outs": [],
                            "sync_info": {"on_update": [],
                                          "on_wait": extra[i:i + cap]},
                        })
                    changed = True
                out.append(inst)
            blk["instructions"] = out
    return orjson.dumps(m) if changed else bir_json


def _install_fix():
    import concourse.bass_utils as bu
    import concourse.bass2jax as b2j
    if getattr(bu, "_wsplit_installed", False):
        return
    orig = bu.compile_bir_kernel

    def wrapped(bir_json, tmpdir, neff_name="file.neff"):
        return orig(_split_sync_waits(bir_json), tmpdir, neff_name)

    bu.compile_bir_kernel = wrapped
    b2j.compile_bir_kernel = wrapped
    bu._wsplit_installed = True


# ---------------------------------------------------------------- device code
def _build_nc():
    import concourse.bass as bass
    import concourse.mybir as mybir
    import concourse.tile as tile
    from concourse.masks import make_identity

    f32 = mybir.dt.float32
    f32r = mybir.dt.float32r
    AX = mybir.AxisListType
    AF = mybir.ActivationFunctionType

    nc = bass.Bass()
    xT = nc.declare_dram_parameter("xT", [DIM, T], f32r, isOutput=False)
    wq = nc.declare_dram_parameter("wq", [DIM, 2048], f32r, isOutput=False)
    halo = nc.declare_dram_parameter("halo", [2048, 3], f32, isOutput=False)
    apeq = nc.declare_dram_parameter("apeq", [1024, 8], f32, isOutput=False)
    cosb = nc.declare_dram_parameter("cosb", [T, 32], f32, isOutput=False)
    sinb = nc.declare_dram_parameter("sinb", [T, 32], f32, isOutput=False)
    nw = nc.declare_dram_parameter("nw", [HD], f32, isOutput=False)
    out_ckv = nc.declare_dram_parameter("out_ckv", [T, HD], f32, isOutput=True)

    with tile.TileContext(nc) as tc, ExitStack() as ctx:
        singles = ctx.enter_context(tc.tile_pool(name="singles", bufs=1))
        wpool = ctx.enter_context(tc.tile_pool(name="wp", bufs=2))
        xpool = ctx.enter_context(tc.tile_pool(name="xp", bufs=2))
        accp = ctx.enter_context(tc.tile_pool(name="accp", bufs=1))
        winp = ctx.enter_context(tc.tile_pool(name="winp", bufs=1))
        ckvp = ctx.enter_context(tc.tile_pool(name="ckvp", bufs=1))
        scr = ctx.enter_context(tc.tile_pool(name="scr", bufs=1))
        pp = ctx.enter_context(tc.tile_pool(name="pp", bufs=4, space="PSUM"))
        ppt = ctx.enter_context(tc.tile_pool(name="ppt", bufs=2, space="PSUM"))

        ident = singles.tile([128, 128], f32)
        make_identity(nc, ident)
        eps_t = singles.tile([128, 1], f32)
        nc.vector.memset(eps_t, EPS)
        nw_b = singles.tile([128, HD], f32)
        nw_ap = nw[:]
        nc.sync.dma_start(out=nw_b, in_=bass.AP(
            tensor=nw_ap.tensor, offset=nw_ap.offset,
            ap=[[0, 128]] + list(nw_ap.ap)))
        cos_t = singles.tile([128, 16, 32], f32)
        nc.sync.dma_start(out=cos_t,
                          in_=cosb[:, :].rearrange("(ts p) c -> p ts c", p=128))
        sin_t = singles.tile([128, 16, 32], f32)
        nc.sync.dma_start(out=sin_t,
                          in_=sinb[:, :].rearrange("(ts p) c -> p ts c", p=128))
        ape_t = singles.tile([128, 8, 8], f32)
        nc.sync.dma_start(out=ape_t,
                          in_=apeq[:, :].rearrange("(bl p) c -> p bl c", p=128))

        ckv_t = ckvp.tile([128, 16, HD], f32)

        for qp in range(2):
            acc = accp.tile([128, 8, ACCW], f32, tag="acc")
            nc.vector.memset(acc[:, :, ACCW - 1:ACCW], 0.0)
            for m8 in range(8):
                nc.sync.dma_start(
                    out=acc[:, m8, 0:3],
                    in_=halo[qp * 1024 + m8 * 128: qp * 1024 + (m8 + 1) * 128, :])
            for kb in range(4):
                for half in range(2):
                    wt = wpool.tile([128, 8, 512], f32r, tag="wt")
                    nc.sync.dma_start(
                        out=wt,
                        in_=wq[kb * 1024:(kb + 1) * 1024,
                               qp * 1024 + half * 512:
                               qp * 1024 + (half + 1) * 512]
                        .rearrange("(ks p) c -> p ks c", p=128))
                    for g in range(4):
                        xt = xpool.tile([128, 8, 512], f32r, tag="xt")
                        nc.sync.dma_start(
                            out=xt,
                            in_=xT[kb * 1024:(kb + 1) * 1024,
                                   g * 512:(g + 1) * 512]
                            .rearrange("(ks p) t -> p ks t", p=128))
                        for m8h in range(4):
                            m8 = half * 4 + m8h
                            ps = pp.tile([128, 512], f32, tag="ps")
                            for k in range(8):
                                nc.tensor.matmul(
                                    ps[:],
                                    wt[:, k, m8h * 128:(m8h + 1) * 128],
                                    xt[:, k, :],
                                    start=(k == 0), stop=(k == 7))
                            dst = acc[:, m8, 3 + g * 512: 3 + (g + 1) * 512]
                            if kb == 0:
                                nc.scalar.copy(out=dst, in_=ps[:])
                            else:
                                nc.vector.tensor_add(out=dst, in0=dst,
                                                     in1=ps[:])
            # window phase, per quad
            for ql in range(2):
                f = 2 * qp + ql
                q4 = ql * 4
                kv_ab = [acc[:, q4 + 0, :], acc[:, q4 + 1, :]]
                e_ab = [acc[:, q4 + 2, :], acc[:, q4 + 3, :]]
                for s_ in e_ab:
                    nc.scalar.activation(out=s_, in_=s_, func=AF.Exp)
                outs = []
                for role in range(2):
                    kv, e = kv_ab[role], e_ab[role]
                    bl = f * 2 + role
                    av = ape_t[:, bl, 1:5]
                    ape_bc = bass.AP(tensor=av.tensor, offset=av.offset,
                                     ap=[list(av.ap[0]), [0, ACCW // 4],
                                         list(av.ap[1])])
                    kv_r = kv.rearrange("p (a b) -> p a b", b=4)
                    nc.vector.tensor_add(out=kv_r, in0=kv_r, in1=ape_bc)
                    nc.vector.tensor_mul(out=kv, in0=kv, in1=e)
                    aw = winp.tile([128, 2050], f32, tag="aw")
                    nc.vector.tensor_add(out=aw, in0=e[:, 0:2050],
                                         in1=e[:, 1:2051])
                    es = winp.tile([128, 2048], f32, tag="es")
                    nc.vector.tensor_add(out=es, in0=aw[:, 0:2048],
                                         in1=aw[:, 2:2050])
                    nc.vector.reciprocal(out=es, in_=es)
                    aw2 = winp.tile([128, 2050], f32, tag="aw")
                    nc.vector.tensor_add(out=aw2, in0=kv[:, 0:2050],
                                         in1=kv[:, 1:2051])
                    gs = winp.tile([128, 2048], f32,
                                   tag=("fold" if role == 0 else "gs"))
                    nc.vector.tensor_add(out=gs, in0=aw2[:, 0:2048],
                                         in1=aw2[:, 2:2050])
                    nc.vector.tensor_mul(out=gs, in0=gs, in1=es)
                    outs.append(gs)
                fq = outs[0]
                nc.vector.tensor_add(out=fq, in0=fq, in1=outs[1])
                for ts in range(16):
                    pt = ppt.tile([128, 128], f32, tag="pt")
                    nc.tensor.transpose(pt[:],
                                        fq[:, ts * 128:(ts + 1) * 128],
                                        ident)
                    nc.any.tensor_copy(
                        out=ckv_t[:, ts, f * 128:(f + 1) * 128], in_=pt[:])

        # RMSNorm + RoPE in token-major layout
        for ts in range(16):
            sq = scr.tile([128, HD], f32, tag="sq")
            nc.vector.tensor_mul(out=sq, in0=ckv_t[:, ts], in1=ckv_t[:, ts])
            var = scr.tile([128, 1], f32, tag="var")
            nc.vector.reduce_sum(out=var, in_=sq, axis=AX.X)
            nc.scalar.activation(out=var, in_=var, func=AF.Sqrt,
                                 bias=eps_t[:], scale=1.0 / HD)
            nc.vector.reciprocal(out=var, in_=var)
            nc.vector.tensor_scalar_mul(ckv_t[:, ts], ckv_t[:, ts], var[:])
            nc.vector.tensor_mul(out=ckv_t[:, ts], in0=ckv_t[:, ts], in1=nw_b)
        # rope on last RD channels (pairs interleaved)
        ev = ckv_t[:, :, HD - RD:HD].rearrange("p t (i two) -> p t i two",
                                               two=2)
        cr = scr.tile([128, 16, RD // 2], f32, tag="cr")
        ci = scr.tile([128, 16, RD // 2], f32, tag="ci")
        tmp = scr.tile([128, 16, RD // 2], f32, tag="tmp")
        nc.vector.tensor_copy(out=cr, in_=ev[:, :, :, 0])
        nc.vector.tensor_copy(out=ci, in_=ev[:, :, :, 1])
        nc.vector.tensor_mul(out=ev[:, :, :, 0], in0=cr, in1=cos_t)
        nc.vector.tensor_mul(out=tmp, in0=ci, in1=sin_t)
        nc.vector.tensor_sub(out=ev[:, :, :, 0], in0=ev[:, :, :, 0], in1=tmp)
        nc.vector.tensor_mul(out=ev[:, :, :, 1], in0=cr, in1=sin_t)
        nc.vector.tensor_mul(out=tmp, in0=ci, in1=cos_t)
        nc.vector.tensor_add(out=ev[:, :, :, 1], in0=ev[:, :, :, 1], in1=tmp)

        nc.sync.dma_start(
            out=out_ckv[:, :].rearrange("(ts p) c -> p ts c", p=128),
            in_=ckv_t)
    return nc


# ---------------------------------------------------------------- runner
def _get_runner():
    """Build nc once and return a cached callable(in_maps) -> per-core dicts.
    Mimics bass2jax.run_bass_via_pjrt's multi-core path but caches the jit."""
    if _RUNNER[0] is not None:
        return _RUNNER[0]
    _install_fix()
    import jax
    import concourse.mybir as mybir
    from concourse import bass2jax
    from jax.sharding import Mesh, PartitionSpec
    from jax.experimental.shard_map import shard_map

    bass2jax.install_neuronx_cc_hook()
    nc = _build_nc()

    part_name = (nc.partition_id_tensor.name
                 if nc.partition_id_tensor else None)
    in_names, out_names, out_avals = [], [], []
    for alloc in nc.m.functions[0].allocations:
        if not isinstance(alloc, mybir.MemoryLocationSet):
            continue
        name = alloc.memorylocations[0].name
        if alloc.kind == "ExternalInput":
            if name != part_name:
                in_names.append(name)
        elif alloc.kind == "ExternalOutput":
            out_names.append(name)
            out_avals.append(jax.core.ShapedArray(
                tuple(alloc.tensor_shape), mybir.dt.np(alloc.dtype)))
    n_params = len(in_names)
    all_names = in_names + out_names
    if part_name is not None:
        all_names = all_names + [part_name]

    def _body(*args):
        operands = list(args)
        if part_name is not None:
            operands.append(bass2jax.partition_id_tensor())
        outs = bass2jax._bass_exec_p.bind(
            *operands,
            out_avals=tuple(out_avals),
            in_names=tuple(all_names),
            out_names=tuple(out_names),
            lowering_input_output_aliases=(),
            sim_require_finite=False,
            sim_require_nnan=False,
            nc=nc,
        )
        return tuple(outs)

    devices = jax.devices()[:NCORES]
    mesh = Mesh(np.asarray(devices), ("core",))
    n_outs = len(out_names)
    sharded = jax.jit(
        shard_map(_body, mesh=mesh,
                  in_specs=(PartitionSpec("core"),) * (n_params + n_outs),
                  out_specs=(PartitionSpec("core"),) * n_outs,
                  check_rep=False),
        keep_unused=True)

    def run(in_maps):
        concat_in = [np.concatenate([np.asarray(m[name]) for m in in_maps],
                                    axis=0) for name in in_names]
        concat_zeros = [np.zeros((NCORES * a.shape[0], *a.shape[1:]), a.dtype)
                        for a in out_avals]
        out_arrs = sharded(*concat_in, *concat_zeros)
        return [
            {name: np.asarray(out_arrs[i]).reshape(
                NCORES, *out_avals[i].shape)[c]
             for i, name in enumerate(out_names)}
            for c in range(NCORES)
        ]

    _RUNNER[0] = run
    return run


# ---------------------------------------------------------------- host side
def _stage_inputs(x, wkv, wgate, ape, nw, cos, sin):
    blocks = []
    for f in range(4):
        blocks += [wkv[:, f * 128:(f + 1) * 128],
                   wkv[:, (f + 4) * 128:(f + 5) * 128],
                   wgate[:, f * 128:(f + 1) * 128],
                   wgate[:, (f + 4) * 128:(f + 5) * 128]]
    wqm = np.ascontiguousarray(np.concatenate(blocks, axis=1))
    apeT = ape.T
    rows = []
    for f in range(4):
        rows += [apeT[f * 128:(f + 1) * 128], apeT[(f + 4) * 128:(f + 5) * 128]]
    apeq = np.ascontiguousarray(
        np.tile(np.concatenate(rows, axis=0), (1, 2))).astype(np.float32)

    in_maps = []
    for core in range(NCORES):
        b, h = divmod(core, 2)
        t0 = h * T
        xTc = np.ascontiguousarray(x[b, t0:t0 + T].T)
        if h == 0:
            haloc = np.zeros((2048, 3), np.float32)
            ridx = (np.arange(2048) // 128) % 4
            haloc[ridx >= 2] = NEG
        else:
            haloc = np.ascontiguousarray(
                (x[b, t0 - 3:t0].astype(np.float32) @ wqm).T)
        in_maps.append(dict(
            xT=xTc, wq=wqm, halo=haloc, apeq=apeq,
            cosb=np.ascontiguousarray(cos[t0:t0 + T]),
            sinb=np.ascontiguousarray(sin[t0:t0 + T]),
            nw=np.ascontiguousarray(nw)))
    return in_maps


def kernel(**inputs):
    x = np.asarray(inputs["x"], np.float32)
    wkv = np.asarray(inputs["wkv_w"], np.float32)
    wgate = np.asarray(inputs["wgate_w"], np.float32)
    ape = np.asarray(inputs["ape"], np.float32)
    nw = np.asarray(inputs["norm_w"], np.float32)
    cos = np.asarray(inputs["cos"], np.float32)
    sin = np.asarray(inputs["sin"], np.float32)
    bo = np.asarray(inputs["block_offsets"])

    in_maps = _stage_inputs(x, wkv, wgate, ape, nw, cos, sin)
    run = _get_runner()
    results = run(in_maps)

    out = np.stack([results[c]["out_ckv"] for c in range(NCORES)])
    ckv_flat = np.ascontiguousarray(out.reshape(B * S, HD))
    sel = out[:, RATIO - 1::RATIO, :].reshape(B, N_COMP, HD)
    cache = np.zeros((B * N_COMP // BLOCK_SIZE, BLOCK_SIZE, HD), np.float32)
    cidx = np.arange(N_COMP)
    blk = bo[:, cidx // BLOCK_SIZE].reshape(-1)
    off = np.tile(cidx % BLOCK_SIZE, B)
    cache[blk, off] = sel.reshape(-1, HD)
    return ckv_flat, cache
